# revision 11
# baseline (speedup 1.0000x reference)
"""TensorE-centric variant: block-diagonal batched matvecs on the PE array.

Sharding: ic 8-way (144 ic/core), all 32 batch elements as moving columns.
Per core: 18 chunks of 8 ic. Superblock (chunk, oc) = 8 (ic,oc) pairs.
Partitions hold (g, od)=128 or (g, id)=64; free dim holds (oc, b)=320.

Per iteration, per chunk:
  s = Ws^T @ out     10 matmuls [128pi,64po] bf16, N=32   -> psum [64, 320]
  r = xn * 1/s       DVE recip + mul                      -> sbuf [64, 320]
  u = Wu^T @ r       10 matmuls [64pi,128po] bf16, N=32   -> psum [128, 320]
  out = out * u      DVE mul (psum operand)               -> sbuf [128, 320]
Epilogue: rec via s-matmuls; alpha/z/partition-broadcast/final sum-over-ic
via constant indicator stationaries; od-normalization folded into the alpha
factor. Host pre-builds block-diagonal stationaries, xn, and 1/rowsum(w).
"""

import numpy as np

B, IC, OC, ID, OD = 32, 1152, 10, 8, 16
N_CORES = 8
IC_LOC = IC // N_CORES        # 144
G = 8                         # ic per chunk
NCH = IC_LOC // G             # 18 chunks
PF = OC * B                   # 320 free (oc-major, b-minor)
EPS = 1e-20
N_ITER = 5

_CACHE = {}


def build_program():
    import concourse.bacc as bacc
    import concourse.tile as tile
    from concourse import mybir
    from concourse.bass import broadcast_tensor_aps

    f32 = mybir.dt.float32
    bf16 = mybir.dt.bfloat16
    X = mybir.AxisListType.X

    nc = bacc.Bacc("TRN2", target_bir_lowering=False, debug=False,
                   enable_asserts=True)

    ws_d = nc.declare_dram_parameter("ws", [NCH, 128, OC * 64], bf16,
                                     isOutput=False)
    wu_d = nc.declare_dram_parameter("wu", [NCH, 64, OC * 128], bf16,
                                     isOutput=False)
    xn_d = nc.declare_dram_parameter("xn", [NCH, 64, B], f32, isOutput=False)
    swr_d = nc.declare_dram_parameter("swr", [NCH, 64, OC], f32,
                                      isOutput=False)
    cst_d = nc.declare_dram_parameter("cst", [128, 160], bf16, isOutput=False)
    out_d = nc.declare_dram_parameter("out", [16, PF], f32, isOutput=True)

    def bmul(out_ap, a_ap, b_ap):
        a2, b2 = broadcast_tensor_aps(a_ap, b_ap)
        nc.vector.tensor_mul(out_ap, a2, b2)

    with tile.TileContext(nc) as tc:
        with (
            tc.tile_pool(name="consts", bufs=1) as constp,
            tc.tile_pool(name="wpool", bufs=1) as wpool,
            tc.tile_pool(name="state", bufs=1) as statep,
            tc.tile_pool(name="work", bufs=3) as workp,
            tc.tile_pool(name="psum", bufs=2, space="PSUM") as psump,
            tc.tile_pool(name="psmall", bufs=1, space="PSUM") as psmallp,
            tc.tile_pool(name="psy", bufs=1, space="PSUM") as psyp,
        ):
            cst = constp.tile([128, 160], bf16)
            nc.sync.dma_start(out=cst[:], in_=cst_d[:])
            onesI16 = cst[:, 0:16]          # [128,16]: (g,od)->od
            onesZ = cst[:, 16:24]           # [128,8]: (g,od)->g
            ones8 = cst[0:64, 24:32]        # [64,8]:  (g,id)->g
            bcast8 = cst[0:8, 32:160]       # [8,128]: g->(g,od)

            y_ps = psyp.tile([16, PF], f32)

            ws_t, wu_t, xn_t, swr_t, out_t = [], [], [], [], []
            for ch in range(NCH):
                wst = wpool.tile([128, OC * 64], bf16, tag=f"ws{ch}",
                                 name=f"ws{ch}")
                nc.sync.dma_start(out=wst[:], in_=ws_d[ch])
                ws_t.append(wst)
                wut = wpool.tile([64, OC * 128], bf16, tag=f"wu{ch}",
                                 name=f"wu{ch}")
                nc.sync.dma_start(out=wut[:], in_=wu_d[ch])
                wu_t.append(wut)
                xnt = statep.tile([64, 1, B], f32, tag=f"xn{ch}",
                                  name=f"xn{ch}")
                nc.sync.dma_start(out=xnt[:, 0, :], in_=xn_d[ch])
                xn_t.append(xnt)
                swrt = statep.tile([64, OC, 1], f32, tag=f"swr{ch}",
                                   name=f"swr{ch}")
                nc.sync.dma_start(out=swrt[:, :, 0], in_=swr_d[ch])
                swr_t.append(swrt)
                out_t.append(statep.tile([128, PF], bf16, tag=f"out{ch}",
                                         name=f"out{ch}"))

            def s_matmuls(ch, dst_ps):
                for oc in range(OC):
                    nc.tensor.matmul(out=dst_ps[:, oc * B:(oc + 1) * B],
                                     lhsT=ws_t[ch][:, oc * 64:(oc + 1) * 64],
                                     rhs=out_t[ch][:, oc * B:(oc + 1) * B])

            for ch in range(NCH):
                # ---- iteration 1: r0 = swr * xn, out = Wu^T @ r0 ----
                r = workp.tile([64, OC, B], bf16, tag="r")
                bmul(r[:], swr_t[ch][:], xn_t[ch][:])
                ps_u = psump.tile([128, PF], f32, tag="psu")
                for oc in range(OC):
                    nc.tensor.matmul(out=ps_u[:, oc * B:(oc + 1) * B],
                                     lhsT=wu_t[ch][0:64,
                                                   oc * 128:(oc + 1) * 128],
                                     rhs=r[:, oc, :])
                nc.scalar.copy(out=out_t[ch][:], in_=ps_u[:])

            for k in range(1, N_ITER):
                for ch in range(NCH):
                    ps_s = psump.tile([64, PF], f32, tag="pss")
                    s_matmuls(ch, ps_s)
                    srec = workp.tile([64, OC, B], f32, tag="srec")
                    nc.vector.reciprocal_approx_fast(
                        out=srec[:].rearrange("p a b -> p (a b)"), in_=ps_s[:])
                    r = workp.tile([64, OC, B], bf16, tag="r")
                    bmul(r[:], srec[:], xn_t[ch][:])
                    ps_u = psump.tile([128, PF], f32, tag="psu")
                    for oc in range(OC):
                        nc.tensor.matmul(out=ps_u[:, oc * B:(oc + 1) * B],
                                         lhsT=wu_t[ch][0:64,
                                                       oc * 128:(oc + 1) * 128],
                                         rhs=r[:, oc, :])
                    nc.vector.tensor_mul(out_t[ch][:], out_t[ch][:], ps_u[:])

            for ch in range(NCH):
                # ---- epilogue ----
                ps_s = psump.tile([64, PF], f32, tag="pss")
                s_matmuls(ch, ps_s)
                recxn = workp.tile([64, OC, B], bf16, tag="recxn")
                bmul(recxn[:], ps_s[:].rearrange("p (a b) -> p a b", a=OC),
                     xn_t[ch][:])
                ps_a = psmallp.tile([8, PF], f32, tag="psa")
                nc.tensor.matmul(out=ps_a[:], lhsT=ones8,
                                 rhs=recxn[:].rearrange("p a b -> p (a b)"))
                ps_z = psmallp.tile([8, PF], f32, tag="psz")
                nc.tensor.matmul(out=ps_z[:], lhsT=onesZ, rhs=out_t[ch][:])
                zrec = workp.tile([8, OC, B], f32, tag="zrec")
                nc.vector.reciprocal_approx_fast(
                    out=zrec[:].rearrange("p a b -> p (a b)"), in_=ps_z[:])
                at = workp.tile([8, OC, B], f32, tag="at")
                nc.vector.tensor_mul(at[:].rearrange("p a b -> p (a b)"),
                                     ps_a[:],
                                     zrec[:].rearrange("p a b -> p (a b)"))
                za = workp.tile([8, 1, B], f32, tag="za")
                nc.vector.reduce_sum(
                    out=za[:, 0, :],
                    in_=at[:].rearrange("p a b -> p b a"), axis=X)
                nc.vector.reciprocal_approx_fast(out=za[:, 0, :],
                                                 in_=za[:, 0, :])
                bmul(at[:], at[:], za[:])
                fac = workp.tile([8, OC, B], bf16, tag="fac")
                nc.vector.tensor_mul(fac[:], at[:], zrec[:])
                ps_f = psump.tile([128, PF], f32, tag="psu")
                nc.tensor.matmul(out=ps_f[:], lhsT=bcast8,
                                 rhs=fac[:].rearrange("p a b -> p (a b)"))
                c = workp.tile([128, PF], bf16, tag="c")
                nc.vector.tensor_mul(c[:], out_t[ch][:], ps_f[:])
                nc.tensor.matmul(out=y_ps[:], lhsT=onesI16, rhs=c[:],
                                 start=(ch == 0), stop=(ch == NCH - 1))

            ostage = constp.tile([16, PF], f32)
            nc.scalar.copy(out=ostage[:], in_=y_ps[:])
            nc.sync.dma_start(out=out_d[:], in_=ostage[:])

    nc.compile()
    return nc


def _get_nc():
    if "nc" not in _CACHE:
        _CACHE["nc"] = build_program()
    return _CACHE["nc"]


def _prep_in_maps(x, weights):
    import ml_dtypes
    bf = ml_dtypes.bfloat16
    x = np.asarray(x, dtype=np.float32)
    w = np.asarray(weights, dtype=np.float32)
    xn = x / (x.sum(-1, keepdims=True) + EPS)        # [B, IC, ID]
    swr = 1.0 / (w.sum(-1) + EPS)                    # [IC, OC, ID]

    cst = np.zeros((128, 160), np.float32)
    for g in range(G):
        cst[g * 16:(g + 1) * 16, 0:16] = np.eye(16)          # onesI16
        cst[g * 16:(g + 1) * 16, 16 + g] = 1.0               # onesZ
        cst[g * 8:(g + 1) * 8, 24 + g] = 1.0                 # ones8 (rows 0:64)
        cst[g, 32 + g * 16:32 + (g + 1) * 16] = 1.0          # bcast8 (rows 0:8)
    cst = cst.astype(bf)

    in_maps = []
    for cidx in range(N_CORES):
        ic0 = cidx * IC_LOC
        wc = w[ic0:ic0 + IC_LOC]                     # [144, OC, ID, OD]
        ws = np.zeros((NCH, 128, OC, 64), np.float32)
        wu = np.zeros((NCH, 64, OC, 128), np.float32)
        xnc = np.zeros((NCH, 64, B), np.float32)
        swrc = np.zeros((NCH, 64, OC), np.float32)
        for ch in range(NCH):
            for g in range(G):
                icg = ch * G + g
                blk = wc[icg]                        # [OC, ID, OD]
                for oc in range(OC):
                    ws[ch, g * 16:(g + 1) * 16, oc, g * 8:(g + 1) * 8] = \
                        blk[oc].T                    # [OD, ID]
                    wu[ch, g * 8:(g + 1) * 8, oc, g * 16:(g + 1) * 16] = \
                        blk[oc]                      # [ID, OD]
                xnc[ch, g * 8:(g + 1) * 8, :] = \
                    xn[:, ic0 + icg, :].T            # [ID, B]
                swrc[ch, g * 8:(g + 1) * 8, :] = \
                    swr[ic0 + icg].T                 # [ID, OC]
        in_maps.append({
            "ws": np.ascontiguousarray(
                ws.reshape(NCH, 128, OC * 64)).astype(bf),
            "wu": np.ascontiguousarray(
                wu.reshape(NCH, 64, OC * 128)).astype(bf),
            "xn": xnc, "swr": swrc, "cst": cst,
        })
    return in_maps


def kernel(x: np.ndarray, weights: np.ndarray) -> np.ndarray:
    from concourse.bass_utils import run_bass_kernel_spmd

    in_maps = _prep_in_maps(x, weights)
    nc = _get_nc()
    results = run_bass_kernel_spmd(nc, in_maps, list(range(N_CORES)))
    _CACHE["last_results"] = results
    return _gather(results.results)


def _gather(res):
    total = np.zeros((16, OC, B), np.float64)
    for c in range(N_CORES):
        total += res[c]["out"].reshape(16, OC, B)
    return np.ascontiguousarray(total.transpose(2, 1, 0)).astype(np.float32)
